# revision 1
# baseline (speedup 1.0000x reference)
"""3-layer GCN (GCNConv+BN+ReLU x2, GCNConv+log_softmax) on 8 trn2 NeuronCores.

Strategy: nodes are degree-sorted and dealt round-robin to 8 cores (balanced
shards). Per layer: each core computes h = act.T @ W for its own nodes (PE),
scales rows by dinv (ACT), writes its shard, AllGather -> full table in DRAM.
Aggregation: edges are packed into dst-aligned slots (slot partition == dst
lane); dma_gather fetches dinv[src]*h[src] rows for 128-edge blocks, identity
matmuls accumulate blocks into PSUM (segment-sum with zero-row padding), and
the epilogue applies dinv[dst], BN+ReLU (folded into one ACT op) after a PE
transpose back to feature-major for the next layer's matmul.  Gather indices
are int16 with a mid-table base so signed offsets cover all 50008 table rows.
"""
import numpy as np

N = 50000
E = 800000
D_IN = 128
D_H = 128
D_OUT = 40
D_OUT_PAD = 64
BN_EPS = 1e-5
NCORES = 8
SHARD = N // NCORES              # 6250
SHARD_ROWS = SHARD + 1           # + trailing zero row
TBL_ROWS = SHARD_ROWS * NCORES   # 50008
NTILES = (SHARD + 127) // 128    # 49
BASE = 32768                     # gather base row (signed int16 offsets)
ZID = (NCORES - 1) * SHARD_ROWS + SHARD  # 50007: a zero row in the hi range


def _preprocess(x, src, dst):
    deg = np.bincount(dst, minlength=N).astype(np.float64) + 1.0
    dinv = (1.0 / np.sqrt(deg)).astype(np.float32)
    order = np.argsort(deg, kind="stable")
    core_of = np.empty(N, np.int64)
    pos_of = np.empty(N, np.int64)
    core_of[order] = np.arange(N) % NCORES
    pos_of[order] = np.arange(N) // NCORES
    tid = core_of * SHARD_ROWS + pos_of          # node -> table row id

    es = np.concatenate([src, np.arange(N)])     # + self loops
    ed = np.concatenate([dst, np.arange(N)])
    ec = core_of[ed]
    ep = pos_of[ed]
    sid_all = tid[es]

    # per-core per-pos counts -> per-tile block counts (max across cores)
    tile_max = np.zeros((NCORES, NTILES), np.int64)
    per_core = []
    for c in range(NCORES):
        sel = ec == c
        pos = ep[sel]
        sid = sid_all[sel]
        o = np.argsort(pos, kind="stable")
        pos, sid = pos[o], sid[o]
        counts = np.bincount(pos, minlength=SHARD)
        cpad = np.zeros(NTILES * 128, np.int64)
        cpad[:SHARD] = counts
        tile_max[c] = cpad.reshape(NTILES, 128).max(1)
        per_core.append((pos, sid, counts))
    blocks = tile_max.max(0)                     # [NTILES]

    # slot arrays: per tile, blocks[t]*128 slots + 16 trailing pad idxs
    slot_off = np.zeros(NTILES, np.int64)
    call_cols = blocks * 8 + 1                   # int16 cols per call (16/col)
    s = 0
    for t in range(NTILES):
        slot_off[t] = s
        s += blocks[t] * 128
    S_slots = int(s)

    idx_wrapped = []
    for c in range(NCORES):
        pos, sid, counts = per_core[c]
        starts = np.concatenate([[0], np.cumsum(counts)[:-1]])
        r = np.arange(len(pos)) - np.repeat(starts, counts)
        tt = pos // 128
        jj = pos % 128
        flat = slot_off[tt] + r * 128 + jj
        slots = np.full(S_slots, ZID, np.int64)
        slots[flat] = sid
        # build wrapped int16 index tile, call by call (16 extra pad idxs each)
        cols = []
        for t in range(NTILES):
            seg = np.full(blocks[t] * 128 + 16, ZID, np.int64)
            seg[:blocks[t] * 128] = slots[slot_off[t]:slot_off[t] + blocks[t] * 128]
            w = (seg - BASE).astype(np.int16).reshape(-1, 16).T  # [16, cols]
            cols.append(w)
        w16 = np.concatenate(cols, axis=1)
        idx_wrapped.append(np.tile(w16, (8, 1)))  # replicate to 128 partitions

    # per-core dinv (node-on-partition per tile) and shard node order
    dinv_own = []
    shard_nodes = []
    for c in range(NCORES):
        nodes = order[c::NCORES]
        shard_nodes.append(nodes)
        dpad = np.zeros(NTILES * 128, np.float32)
        dpad[:SHARD] = dinv[nodes]
        dinv_own.append(dpad.reshape(NTILES, 128).T.copy())  # [128, NTILES]
    return blocks, call_cols, idx_wrapped, dinv_own, shard_nodes


def _build(blocks, call_cols):
    import concourse.bass as bass
    import concourse.tile as tile
    from concourse import bacc, mybir

    f32 = mybir.dt.float32
    nc = bacc.Bacc("TRN2", num_devices=NCORES, debug=False, num_swdge_queues=4)
    SC = int(call_cols.sum())
    xT_in = nc.dram_tensor("xT", [128, SHARD], f32, kind="ExternalInput")
    idx_in = nc.dram_tensor("idx", [128, SC], mybir.dt.int16, kind="ExternalInput")
    dinv_in = nc.dram_tensor("dinvown", [128, NTILES], f32, kind="ExternalInput")
    W1_in = nc.dram_tensor("W1", [128, D_H], f32, kind="ExternalInput")
    W2_in = nc.dram_tensor("W2", [128, D_H], f32, kind="ExternalInput")
    W3_in = nc.dram_tensor("W3", [128, D_OUT_PAD], f32, kind="ExternalInput")
    sb1_in = nc.dram_tensor("sb1", [128, 2], f32, kind="ExternalInput")
    sb2_in = nc.dram_tensor("sb2", [128, 2], f32, kind="ExternalInput")
    b3_in = nc.dram_tensor("b3rep", [128, D_OUT_PAD], f32, kind="ExternalInput")
    id_in = nc.dram_tensor("ident", [128, 128], f32, kind="ExternalInput")
    y_out = nc.dram_tensor("y", [SHARD, D_OUT], f32, kind="ExternalOutput")

    with tile.TileContext(nc) as tc:
        with tc.tile_pool(name="cst", bufs=1) as cst, \
             tc.tile_pool(name="act", bufs=1) as actp, \
             tc.tile_pool(name="wrk", bufs=3) as wrk, \
             tc.tile_pool(name="gb", bufs=3) as gb, \
             tc.tile_pool(name="ps", bufs=2, space="PSUM") as ps, \
             tc.tile_pool(name="dram", bufs=1, space="DRAM") as dram:

            idx_sb = cst.tile([128, SC], mybir.dt.int16)
            nc.sync.dma_start(idx_sb[:], idx_in[:, :])
            dinv_sb = cst.tile([128, NTILES], f32)
            nc.sync.dma_start(dinv_sb[:], dinv_in[:, :])
            W1 = cst.tile([128, D_H], f32)
            nc.sync.dma_start(W1[:], W1_in[:, :])
            W2 = cst.tile([128, D_H], f32)
            nc.sync.dma_start(W2[:], W2_in[:, :])
            W3 = cst.tile([128, D_OUT_PAD], f32)
            nc.sync.dma_start(W3[:], W3_in[:, :])
            sb1 = cst.tile([128, 2], f32)
            nc.sync.dma_start(sb1[:], sb1_in[:, :])
            sb2 = cst.tile([128, 2], f32)
            nc.sync.dma_start(sb2[:], sb2_in[:, :])
            b3r = cst.tile([128, D_OUT_PAD], f32)
            nc.sync.dma_start(b3r[:], b3_in[:, :])
            ident = cst.tile([128, 128], f32)
            nc.sync.dma_start(ident[:], id_in[:, :])
            identb = cst.tile([128, 128], mybir.dt.bfloat16)
            nc.vector.tensor_copy(identb[:], ident[:])
            zrow = cst.tile([128, 128], f32)
            nc.vector.memset(zrow[:], 0.0)
            zrowb = cst.tile([128, 128], mybir.dt.bfloat16)
            nc.vector.memset(zrowb[:], 0.0)

            actA = actp.tile([128, NTILES * 128], f32, tag="actA")
            actB = actp.tile([128, NTILES * 128], f32, tag="actB")
            nc.sync.dma_start(actA[:, :SHARD], xT_in[:, :])

            shards = []
            tables = []
            for l, fo in ((0, D_H), (1, D_H), (2, D_OUT_PAD)):
                tdt = mybir.dt.bfloat16 if l < 2 else f32
                sh = dram.tile([SHARD_ROWS, fo], tdt, tag=f"shard{l}")
                tb = dram.tile([TBL_ROWS, fo], tdt, tag=f"table{l}",
                               addr_space="Shared")
                shards.append(sh)
                tables.append(tb)

            col16 = np.zeros(NTILES + 1, np.int64)
            col16[1:] = np.cumsum(call_cols)
            max_blk = int(blocks.max())

            for l in range(3):
                fo = D_H if l < 2 else D_OUT_PAD
                tdt = mybir.dt.bfloat16 if l < 2 else f32
                idT = identb if l < 2 else ident
                zr = zrowb if l < 2 else zrow
                W = (W1, W2, W3)[l]
                act_in = (actA, actB, actA)[l]
                act_next = (actB, actA, None)[l]
                sh, tb = shards[l], tables[l]

                # phase A: h = act.T @ W per own tile, scaled by dinv[node]
                for t in range(NTILES):
                    pt = min(128, SHARD - t * 128)
                    ph = ps.tile([128, fo], f32, tag="ph")
                    nc.tensor.matmul(ph[:pt, :], lhsT=act_in[:, t * 128:t * 128 + pt],
                                     rhs=W[:], start=True, stop=True)
                    hsb = wrk.tile([128, fo], tdt, tag="hsb")
                    nc.scalar.activation(hsb[:pt, :], ph[:pt, :],
                                         mybir.ActivationFunctionType.Copy,
                                         scale=dinv_sb[:pt, t:t + 1])
                    nc.sync.dma_start(sh[t * 128:t * 128 + pt, :], hsb[:pt, :])
                nc.sync.dma_start(sh[SHARD:SHARD + 1, :], zr[0:1, :fo])

                # phase B: AllGather shard -> table
                nc.gpsimd.collective_compute(
                    "AllGather", mybir.AluOpType.bypass,
                    replica_groups=[list(range(NCORES))],
                    ins=[sh.opt()], outs=[tb.opt()])

                # phase C: gather + segment-sum + epilogue per dst tile
                for t in range(NTILES):
                    nb = int(blocks[t])
                    pt = min(128, SHARD - t * 128)
                    gt = gb.tile([128, max_blk + 1, fo], tdt, tag="g")
                    nc.gpsimd.dma_gather(
                        out_ap=gt[:, :nb + 1, :],
                        in_ap=tb[BASE:, :],
                        idxs_ap=idx_sb[:, col16[t]:col16[t + 1]],
                        num_idxs=nb * 128 + 16,
                        num_idxs_reg=nb * 128 + 16,
                        elem_size=fo,
                        single_packet=False,
                        queue_num=t % 4,
                    )
                    pa = ps.tile([128, fo], f32, tag="pa")
                    for b in range(nb):
                        nc.tensor.matmul(pa[:], lhsT=idT[:], rhs=gt[:, b, :],
                                         start=(b == 0), stop=(b == nb - 1))
                    if l < 2:
                        sbv = (sb1, sb2)[l]
                        zt = wrk.tile([128, 128], f32, tag="zt")
                        nc.scalar.activation(zt[:], pa[:],
                                             mybir.ActivationFunctionType.Copy,
                                             scale=dinv_sb[:, t:t + 1])
                        pT = ps.tile([128, 128], f32, tag="pT")
                        nc.tensor.transpose(pT[:], zt[:], ident[:])
                        nc.scalar.activation(act_next[:, t * 128:(t + 1) * 128], pT[:],
                                             mybir.ActivationFunctionType.Relu,
                                             bias=sbv[:, 1:2], scale=sbv[:, 0:1])
                    else:
                        zt = wrk.tile([128, D_OUT_PAD], f32, tag="zt3")
                        nc.scalar.activation(zt[:], pa[:],
                                             mybir.ActivationFunctionType.Copy,
                                             scale=dinv_sb[:, t:t + 1])
                        nc.vector.tensor_tensor(zt[:], zt[:], b3r[:],
                                                op=mybir.AluOpType.add)
                        mx = wrk.tile([128, 1], f32, tag="mx")
                        nc.vector.tensor_reduce(mx[:], zt[:, :D_OUT],
                                                axis=mybir.AxisListType.X,
                                                op=mybir.AluOpType.max)
                        nmx = wrk.tile([128, 1], f32, tag="nmx")
                        nc.vector.tensor_scalar_mul(nmx[:], mx[:], -1.0)
                        ex = wrk.tile([128, D_OUT], f32, tag="ex")
                        se = wrk.tile([128, 1], f32, tag="se")
                        nc.scalar.activation(ex[:], zt[:, :D_OUT],
                                             mybir.ActivationFunctionType.Exp,
                                             bias=nmx[:, 0:1], accum_out=se[:, 0:1])
                        lse = wrk.tile([128, 1], f32, tag="lse")
                        nc.scalar.activation(lse[:], se[:],
                                             mybir.ActivationFunctionType.Ln)
                        ot = wrk.tile([128, D_OUT], f32, tag="ot")
                        nc.vector.tensor_scalar(ot[:], zt[:, :D_OUT],
                                                scalar1=mx[:, 0:1],
                                                scalar2=lse[:, 0:1],
                                                op0=mybir.AluOpType.subtract,
                                                op1=mybir.AluOpType.subtract)
                        nc.sync.dma_start(y_out[t * 128:t * 128 + pt, :], ot[:pt, :])
    nc.compile()
    return nc


def prepare(x, src, dst, W1, b1, W2, b2, W3, b3,
            g1, be1, m1, v1, g2, be2, m2, v2):
    x = np.asarray(x, np.float32)
    src = np.asarray(src, np.int64)
    dst = np.asarray(dst, np.int64)
    blocks, call_cols, idx_wrapped, dinv_own, shard_nodes = _preprocess(x, src, dst)
    nc = _build(blocks, call_cols)

    s1 = np.asarray(g1, np.float32) / np.sqrt(np.asarray(v1, np.float32) + BN_EPS)
    bias1 = np.asarray(b1, np.float32) * s1 + (np.asarray(be1, np.float32)
                                               - np.asarray(m1, np.float32) * s1)
    s2 = np.asarray(g2, np.float32) / np.sqrt(np.asarray(v2, np.float32) + BN_EPS)
    bias2 = np.asarray(b2, np.float32) * s2 + (np.asarray(be2, np.float32)
                                               - np.asarray(m2, np.float32) * s2)
    sb1 = np.stack([s1, bias1], 1).astype(np.float32)
    sb2 = np.stack([s2, bias2], 1).astype(np.float32)
    W3p = np.zeros((128, D_OUT_PAD), np.float32)
    W3p[:, :D_OUT] = np.asarray(W3, np.float32)
    b3p = np.zeros(D_OUT_PAD, np.float32)
    b3p[:D_OUT] = np.asarray(b3, np.float32)
    b3rep = np.tile(b3p[None, :], (128, 1))
    ident = np.eye(128, dtype=np.float32)

    in_maps = []
    for c in range(NCORES):
        in_maps.append({
            "xT": x[shard_nodes[c]].T.copy(),
            "idx": idx_wrapped[c],
            "dinvown": dinv_own[c],
            "W1": np.asarray(W1, np.float32), "W2": np.asarray(W2, np.float32),
            "W3": W3p, "sb1": sb1, "sb2": sb2, "b3rep": b3rep, "ident": ident,
        })
    return nc, in_maps, shard_nodes


def kernel(**inputs):
    from concourse.bass_utils import run_bass_kernel_spmd

    nc, in_maps, shard_nodes = prepare(**inputs)
    res = run_bass_kernel_spmd(nc, in_maps, core_ids=list(range(NCORES)))
    out = np.zeros((N, D_OUT), np.float32)
    for c in range(NCORES):
        out[shard_nodes[c]] = res.results[c]["y"]
    return out



# revision 30
# speedup vs baseline: 1.1782x; 1.1782x over previous
"""3-layer GCN (GCNConv+BN+ReLU x2, GCNConv+log_softmax) on 8 trn2 NeuronCores.

Strategy (v2): aggregate in input space, transform after. Nodes are
in-degree-sorted and dealt round-robin to 8 cores. Tables T_l hold
h_l(n)*dinv[n] in bf16, node-major (T0 = x*dinv precomputed on host, so
layer 1 needs no collective). Per layer, each core runs a few BIG
transpose-mode dma_gather calls (feature-major output, j-major slot packing
per 128-dst tile), a DVE tensor_reduce per tile for the segment sum, one
PE matmul agg'@W' (+ rank-1 bias matmul via 1/dinv row), and one fused ACT
(relu, scale=dinv^2) producing the next table row, written to the shard.
AllGathers are chunked (4 per layer) and dispatched two gather-calls late
so the CC engine overlaps them with remaining gathers; only the small last
chunk is exposed at the layer boundary. Gather indices are int16 signed
offsets around a mid-table BASE (HW sign-extends); every call is tail-padded
with one block of positive zero-row indices so the trailing-negative drop
rule never bites. The runtime is descriptor-generation-bound on GPSIMD, so
everything else is engineered to hide under it.
"""
import numpy as np

N = 50000
E = 800000
D = 128
D_OUT = 40
D_OUT_PAD = 64
BN_EPS = 1e-5
NCORES = 8
SHARD = N // NCORES              # 6250
SR = 6256                        # shard rows (padded)
TBL = SR * NCORES                # 50048
NTILES = (SHARD + 127) // 128    # 49
BASE = 32768
CAP = 3072                       # max real idxs per gather call
# AllGather chunking (in tiles). The chunk whose table range contains BASE
# (rows 32768..) must be dispatched LAST so the gather's dep on it implies
# all earlier chunks completed (CC queue is in-order). Chunk 2 starts at
# table row 8*4096 = 32768 by construction.
CHUNK_TILES = (16, 16, 2, 11, 4)
NCHUNK = len(CHUNK_TILES)
BASE_CHUNK = 2
# tile processing order: BASE_CHUNK's tiles go last, so its AG is the only
# one exposed at the layer boundary (all other chunk AGs overlap gathers)
def _tile_order():
    b = np.cumsum((0,) + CHUNK_TILES)
    chunks = [list(range(b[i], b[i + 1])) for i in range(NCHUNK)]
    order = []
    for i in range(NCHUNK):
        if i != BASE_CHUNK:
            order += chunks[i]
    order += chunks[BASE_CHUNK]
    return order, [set(c) for c in chunks]


def _chunk_layout():
    b = np.cumsum((0,) + CHUNK_TILES)          # tile bounds, b[-1] == 49
    p = [int(min(x * 128, SR)) for x in b]
    p[-1] = SR                                  # last chunk includes pad rows
    L = [p[i + 1] - p[i] for i in range(NCHUNK)]
    tbl_base = np.concatenate([[0], np.cumsum([NCORES * x for x in L])])
    assert tbl_base[BASE_CHUNK] == BASE
    return p, L, tbl_base


def _preprocess(src, dst):
    import os
    global CAP
    CAP = int(os.environ.get("KERNEL_CAP", CAP))
    p, L, tbl_base = _chunk_layout()
    deg = np.bincount(dst, minlength=N).astype(np.float64) + 1.0
    dinv = (1.0 / np.sqrt(deg)).astype(np.float32)
    order = np.argsort(deg, kind="stable")
    core_of = np.empty(N, np.int64)
    pos_of = np.empty(N, np.int64)
    core_of[order] = np.arange(N) % NCORES
    pos_of[order] = np.arange(N) // NCORES

    pb = np.array(p[1:])                       # chunk upper pos bounds
    def tid_cp(c, pos):
        k = np.searchsorted(pb - 1, pos)       # chunk of pos
        k = np.minimum(k, NCHUNK - 1)
        Lk = np.array(L)[k]
        return tbl_base[k] + c * Lk + (pos - np.array(p[:NCHUNK])[k])

    tid = tid_cp(core_of, pos_of)              # node -> table row
    zid = int(tbl_base[NCHUNK - 1] + (NCORES - 1) * L[NCHUNK - 1]
              + (SR - p[NCHUNK - 1] - 1))
    assert zid == TBL - 1

    es = np.concatenate([src, np.arange(N)])   # + self loops
    ed = np.concatenate([dst, np.arange(N)])
    sid_all = tid[es]
    ec = core_of[ed]
    ep = pos_of[ed]

    counts = np.zeros((NCORES, NTILES * 128), np.int64)
    np.add.at(counts, (ec, ep), 1)
    nb = counts.reshape(NCORES, NTILES, 128).max(axis=(0, 2))  # [NTILES]

    # greedy call grouping over the permuted tile order
    order_t, _ = _tile_order()
    calls = []                                 # (tile_list, nidx)
    cur, acc = [], 0
    for t in order_t:
        w = int(128 * nb[t])
        if acc and acc + w > CAP:
            calls.append((cur, acc))
            cur, acc = [], 0
        cur.append(t)
        acc += w
    calls.append((cur, acc))

    Sl = int((128 * nb).sum())
    tile_off = np.concatenate([[0], np.cumsum(128 * nb)])

    idx_wrapped = []
    dinv_t, dinv2_t, invd_row = [], [], []
    shard_nodes = []
    for c in range(NCORES):
        sel = ec == c
        pos = ep[sel]
        s = sid_all[sel]
        o = np.argsort(pos, kind="stable")
        pos, s = pos[o], s[o]
        cnt = np.bincount(pos, minlength=NTILES * 128)
        starts = np.concatenate([[0], np.cumsum(cnt)[:-1]])
        r = np.arange(len(pos)) - starts[pos]
        t_of = pos // 128
        jj = pos % 128
        flat = tile_off[t_of] + jj * nb[t_of] + r
        slots = np.full(Sl, zid, np.int64)
        slots[flat] = s
        stream = []
        for (tlist, nidx) in calls:
            for t in tlist:
                stream.append(slots[tile_off[t]:tile_off[t] + 128 * nb[t]])
            stream.append(np.full(128, zid, np.int64))  # positive tail pad
        arr = np.concatenate(stream)
        idx16 = (arr - BASE).astype(np.int16)
        w16 = idx16.reshape(-1, 16).T
        idx_wrapped.append(np.tile(w16, (8, 1)).copy())

        nodes = order[c::NCORES]               # pos-ordered own nodes
        shard_nodes.append(nodes)
        dv = np.ones(NTILES * 128, np.float32)
        dv[:SHARD] = dinv[nodes]
        dinv_t.append(dv.reshape(NTILES, 128).T.copy())
        dinv2_t.append((dv * dv).reshape(NTILES, 128).T.copy())
        invd_row.append((1.0 / dv)[None, :].copy())
    return (dinv, tid, nb, calls, idx_wrapped, dinv_t, dinv2_t, invd_row,
            shard_nodes, p, tbl_base, zid)


def _build(nb, calls, p, tbl_base):
    import os
    import concourse.bass as bass
    import concourse.tile as tile
    from concourse import bacc, mybir
    NO_AG = bool(int(os.environ.get("KERNEL_NO_AG", "0")))
    NLAYERS = int(os.environ.get("KERNEL_NLAYERS", "3"))

    f32 = mybir.dt.float32
    bf16 = mybir.dt.bfloat16
    i16 = mybir.dt.int16
    AF = mybir.ActivationFunctionType
    nc = bacc.Bacc("TRN2", num_devices=NCORES, debug=False,
                   num_swdge_queues=4, dynamic_dma_scratch_size=32768)

    SC = sum((nidx + 128) // 16 for (_, nidx) in calls)
    GMAX = max(nidx for (_, nidx) in calls) + 128
    t0_in = nc.dram_tensor("t0", [TBL, D], bf16, kind="ExternalInput")
    idx_in = nc.dram_tensor("idx", [128, SC], i16, kind="ExternalInput")
    w1_in = nc.dram_tensor("w1", [128, D], f32, kind="ExternalInput")
    w2_in = nc.dram_tensor("w2", [128, D], f32, kind="ExternalInput")
    w3_in = nc.dram_tensor("w3", [128, D_OUT_PAD], f32, kind="ExternalInput")
    b1_in = nc.dram_tensor("b1r", [1, D], f32, kind="ExternalInput")
    b2_in = nc.dram_tensor("b2r", [1, D], f32, kind="ExternalInput")
    b3_in = nc.dram_tensor("b3r", [1, D_OUT_PAD], f32, kind="ExternalInput")
    invd_in = nc.dram_tensor("invd", [1, NTILES * 128], f32,
                             kind="ExternalInput")
    dinv_in = nc.dram_tensor("dinvt", [128, NTILES], f32, kind="ExternalInput")
    dinv2_in = nc.dram_tensor("dinv2t", [128, NTILES], f32,
                              kind="ExternalInput")
    y_out = nc.dram_tensor("y", [SHARD, D_OUT], f32, kind="ExternalOutput")

    with tile.TileContext(nc) as tc:
        with tc.tile_pool(name="cst", bufs=1) as cst, \
             tc.tile_pool(name="gp", bufs=4) as gp, \
             tc.tile_pool(name="wrk", bufs=4) as wrk, \
             tc.tile_pool(name="ps", bufs=4, space="PSUM") as ps, \
             tc.tile_pool(name="dram", bufs=1, space="DRAM") as dram:

            idx_sb = cst.tile([128, SC], i16)
            nc.sync.dma_start(idx_sb[:], idx_in[:, :])
            w1s = cst.tile([128, D], f32)
            nc.sync.dma_start(w1s[:], w1_in[:, :])
            w2s = cst.tile([128, D], f32)
            nc.sync.dma_start(w2s[:], w2_in[:, :])
            w3s = cst.tile([128, D_OUT_PAD], f32)
            nc.sync.dma_start(w3s[:], w3_in[:, :])
            b1s = cst.tile([1, D], f32)
            nc.sync.dma_start(b1s[:], b1_in[:, :])
            b2s = cst.tile([1, D], f32)
            nc.sync.dma_start(b2s[:], b2_in[:, :])
            b3s = cst.tile([1, D_OUT_PAD], f32)
            nc.sync.dma_start(b3s[:], b3_in[:, :])
            invd_sb = cst.tile([1, NTILES * 128], f32)
            nc.sync.dma_start(invd_sb[:], invd_in[:, :])
            dinv_sb = cst.tile([128, NTILES], f32)
            nc.sync.dma_start(dinv_sb[:], dinv_in[:, :])
            dinv2_sb = cst.tile([128, NTILES], f32)
            nc.sync.dma_start(dinv2_sb[:], dinv2_in[:, :])
            zrow = cst.tile([128, D], bf16)
            nc.vector.memset(zrow[:], 0.0)

            sh1 = dram.tile([SR, D], bf16, tag="sh1")
            sh2 = dram.tile([SR, D], bf16, tag="sh2")
            # chunked tables: contiguous Shared tensors, one AG writer each
            tchunks = []
            for ln in (1, 2):
                cs = [nc.dram_tensor(f"tb{ln}c{k}",
                                     [NCORES * (p[k + 1] - p[k]), D], bf16,
                                     addr_space="Shared")
                      for k in range(NCHUNK)]
                a0 = nc.lookup_mls(cs[0]).memorylocations[0].addr
                for k in range(1, NCHUNK):
                    ak = nc.lookup_mls(cs[k]).memorylocations[0].addr
                    exp = a0 + tbl_base[k] * D * 2
                    assert ak == exp, (ln, k, ak, exp)
                tchunks.append(cs)

            _, chunk_sets = _tile_order()
            chunk_of_tile = {}
            for ci, cs in enumerate(chunk_sets):
                for t in cs:
                    chunk_of_tile[t] = ci
            rg = [list(range(NCORES))]

            def emit_ag(sh, chunks, ck):
                if NO_AG:
                    return
                nc.gpsimd.collective_compute(
                    "AllGather", mybir.AluOpType.bypass,
                    replica_groups=rg,
                    ins=[sh[p[ck]:p[ck + 1], :].opt()],
                    outs=[chunks[ck][:, :].opt()])

            for l in range(NLAYERS):
                fo = D if l < 2 else D_OUT_PAD
                W = (w1s, w2s, w3s)[l]
                br = (b1s, b2s, b3s)[l]
                src = (t0_in[BASE:, :] if l == 0
                       else tchunks[l - 1][BASE_CHUNK][:, :])
                sh = (sh1, sh2, None)[l]
                ch_next = (tchunks[0], tchunks[1], None)[l]
                pend_ag = []          # (dispatch_after_call, chunk_idx)
                done_ag = []
                col = 0
                done = [0] * NCHUNK   # tiles emitted per chunk
                for k, (tlist, nidx) in enumerate(calls):
                    nid = nidx + 128
                    gout = gp.tile([128, 1, GMAX], bf16, tag="g")
                    nc.gpsimd.dma_gather(
                        out_ap=gout[:, :, :nid],
                        in_ap=src,
                        idxs_ap=idx_sb[:, col:col + nid // 16],
                        num_idxs=nid, num_idxs_reg=nid, elem_size=D,
                        transpose=True, single_packet=False, queue_num=k % 4)
                    col += nid // 16
                    off = 0
                    for t in tlist:
                        nbt = int(nb[t])
                        pt = min(128, SHARD - t * 128)
                        agg = wrk.tile([128, 128], f32, tag="agg")
                        v = gout[:, 0, off:off + 128 * nbt].rearrange(
                            "q (j b) -> q j b", b=nbt)
                        nc.vector.tensor_reduce(agg[:], v,
                                                axis=mybir.AxisListType.X,
                                                op=mybir.AluOpType.add)
                        off += 128 * nbt
                        pst = ps.tile([128, fo], f32, tag="ps")
                        nc.tensor.matmul(pst[:], lhsT=agg[:], rhs=W[:],
                                         start=True, stop=False)
                        nc.tensor.matmul(
                            pst[:],
                            lhsT=invd_sb[0:1, t * 128:(t + 1) * 128],
                            rhs=br[0:1, :], start=False, stop=True)
                        if l < 2:
                            hsb = wrk.tile([128, D], bf16, tag="hsb")
                            nc.scalar.activation(hsb[:], pst[:], AF.Relu,
                                                 scale=dinv2_sb[:, t:t + 1])
                            nc.sync.dma_start(sh[t * 128:t * 128 + pt, :],
                                              hsb[:pt, :])
                            if t == NTILES - 1:
                                nc.sync.dma_start(sh[SHARD:SR, :],
                                                  zrow[0:SR - SHARD, :])
                            ck = chunk_of_tile[t]
                            done[ck] += 1
                            if (done[ck] == CHUNK_TILES[ck]
                                    and ck != BASE_CHUNK):
                                pend_ag.append((k + 1, ck))
                        else:
                            zt = wrk.tile([128, D_OUT_PAD], f32, tag="zt")
                            nc.scalar.activation(zt[:], pst[:], AF.Copy,
                                                 scale=dinv_sb[:, t:t + 1])
                            mx = wrk.tile([128, 1], f32, tag="mx")
                            nc.vector.tensor_reduce(mx[:], zt[:, :D_OUT],
                                                    axis=mybir.AxisListType.X,
                                                    op=mybir.AluOpType.max)
                            nmx = wrk.tile([128, 1], f32, tag="nmx")
                            nc.vector.tensor_scalar_mul(nmx[:], mx[:], -1.0)
                            ex = wrk.tile([128, D_OUT], f32, tag="ex")
                            se = wrk.tile([128, 1], f32, tag="se")
                            nc.scalar.activation(ex[:], zt[:, :D_OUT],
                                                 AF.Exp, bias=nmx[:, 0:1],
                                                 accum_out=se[:, 0:1])
                            lse = wrk.tile([128, 1], f32, tag="lse")
                            nc.scalar.activation(lse[:], se[:], AF.Ln)
                            ot = wrk.tile([128, D_OUT], f32, tag="ot")
                            nc.vector.tensor_scalar(
                                ot[:], zt[:, :D_OUT],
                                scalar1=mx[:, 0:1], scalar2=lse[:, 0:1],
                                op0=mybir.AluOpType.subtract,
                                op1=mybir.AluOpType.subtract)
                            nc.sync.dma_start(y_out[t * 128:t * 128 + pt, :],
                                              ot[:pt, :])
                    # after this call: dispatch chunk AGs whose writes are
                    # done and which have aged one call (no gpsimd stall)
                    while pend_ag and pend_ag[0][0] <= k:
                        emit_ag(sh, ch_next, pend_ag.pop(0)[1])
                # flush remaining chunks, BASE_CHUNK strictly last
                if l < 2:
                    for (_, ck) in pend_ag:
                        emit_ag(sh, ch_next, ck)
                    emit_ag(sh, ch_next, BASE_CHUNK)
    nc.compile()
    return nc


def prepare(x, src, dst, W1, b1, W2, b2, W3, b3,
            g1, be1, m1, v1, g2, be2, m2, v2):
    import ml_dtypes
    bf = ml_dtypes.bfloat16
    x = np.asarray(x, np.float32)
    src = np.asarray(src, np.int64)
    dst = np.asarray(dst, np.int64)
    (dinv, tid, nb, calls, idx_wrapped, dinv_t, dinv2_t, invd_row,
     shard_nodes, p, tbl_base, zid) = _preprocess(src, dst)
    nc = _build(nb, calls, p, tbl_base)

    s1 = np.asarray(g1, np.float32) / np.sqrt(np.asarray(v1, np.float32)
                                              + BN_EPS)
    s2 = np.asarray(g2, np.float32) / np.sqrt(np.asarray(v2, np.float32)
                                              + BN_EPS)
    w1p = np.asarray(W1, np.float32) * s1[None, :]
    w2p = np.asarray(W2, np.float32) * s2[None, :]
    b1p = ((np.asarray(b1, np.float32) - np.asarray(m1, np.float32)) * s1
           + np.asarray(be1, np.float32))[None, :]
    b2p = ((np.asarray(b2, np.float32) - np.asarray(m2, np.float32)) * s2
           + np.asarray(be2, np.float32))[None, :]
    w3p = np.zeros((128, D_OUT_PAD), np.float32)
    w3p[:, :D_OUT] = np.asarray(W3, np.float32)
    b3p = np.zeros((1, D_OUT_PAD), np.float32)
    b3p[0, :D_OUT] = np.asarray(b3, np.float32)

    t0 = np.zeros((TBL, D), np.float32)
    t0[tid] = x * dinv[:, None]
    t0 = t0.astype(bf)

    in_maps = []
    for c in range(NCORES):
        in_maps.append({
            "t0": t0, "idx": idx_wrapped[c],
            "w1": w1p, "w2": w2p, "w3": w3p,
            "b1r": b1p, "b2r": b2p, "b3r": b3p,
            "invd": invd_row[c],
            "dinvt": dinv_t[c], "dinv2t": dinv2_t[c],
        })
    return nc, in_maps, shard_nodes


def kernel(**inputs):
    from concourse.bass_utils import run_bass_kernel_spmd

    nc, in_maps, shard_nodes = prepare(**inputs)
    res = run_bass_kernel_spmd(nc, in_maps, core_ids=list(range(NCORES)))
    out = np.zeros((N, D_OUT), np.float32)
    for c in range(NCORES):
        out[shard_nodes[c][:SHARD]] = res.results[c]["y"]
    return out


# revision 34
# speedup vs baseline: 1.2113x; 1.0281x over previous
"""3-layer GCN (GCNConv+BN+ReLU x2, GCNConv+log_softmax) on 8 trn2 NeuronCores.

Strategy (v2): aggregate in input space, transform after. Nodes are
in-degree-sorted and dealt round-robin to 8 cores. Tables T_l hold
h_l(n)*dinv[n] in bf16, node-major (T0 = x*dinv precomputed on host, so
layer 1 needs no collective). Per layer, each core runs a few BIG
transpose-mode dma_gather calls (feature-major output, j-major slot packing
per 128-dst tile), a DVE tensor_reduce per tile for the segment sum, one
PE matmul agg'@W' (+ rank-1 bias matmul via 1/dinv row), and one fused ACT
(relu, scale=dinv^2) producing the next table row, written to the shard.
AllGathers are chunked (4 per layer) and dispatched two gather-calls late
so the CC engine overlaps them with remaining gathers; only the small last
chunk is exposed at the layer boundary. Gather indices are int16 signed
offsets around a mid-table BASE (HW sign-extends); every call is tail-padded
with one block of positive zero-row indices so the trailing-negative drop
rule never bites. The runtime is descriptor-generation-bound on GPSIMD, so
everything else is engineered to hide under it.
"""
import numpy as np

N = 50000
E = 800000
D = 128
D_OUT = 40
D_OUT_PAD = 64
BN_EPS = 1e-5
NCORES = 8
SHARD = N // NCORES              # 6250
SR = 6256                        # shard rows (padded)
TBL = SR * NCORES                # 50048
NTILES = (SHARD + 127) // 128    # 49
BASE = 32768
CAP = 3072                       # max real idxs per gather call
# AllGather chunking (in tiles). The chunk whose table range contains BASE
# (rows 32768..) must be dispatched LAST so the gather's dep on it implies
# all earlier chunks completed (CC queue is in-order). Chunk 2 starts at
# table row 8*4096 = 32768 by construction.
CHUNK_TILES = (16, 16, 2, 11, 4)
NCHUNK = len(CHUNK_TILES)
BASE_CHUNK = 2
# tile processing order: BASE_CHUNK's tiles go last, so its AG is the only
# one exposed at the layer boundary (all other chunk AGs overlap gathers)
def _tile_order():
    b = np.cumsum((0,) + CHUNK_TILES)
    chunks = [list(range(b[i], b[i + 1])) for i in range(NCHUNK)]
    order = []
    for i in range(NCHUNK):
        if i != BASE_CHUNK:
            order += chunks[i]
    order += chunks[BASE_CHUNK]
    return order, [set(c) for c in chunks]


def _chunk_layout():
    b = np.cumsum((0,) + CHUNK_TILES)          # tile bounds, b[-1] == 49
    p = [int(min(x * 128, SR)) for x in b]
    p[-1] = SR                                  # last chunk includes pad rows
    L = [p[i + 1] - p[i] for i in range(NCHUNK)]
    tbl_base = np.concatenate([[0], np.cumsum([NCORES * x for x in L])])
    assert tbl_base[BASE_CHUNK] == BASE
    return p, L, tbl_base


def _preprocess(src, dst):
    import os
    global CAP
    CAP = int(os.environ.get("KERNEL_CAP", CAP))
    p, L, tbl_base = _chunk_layout()
    deg = np.bincount(dst, minlength=N).astype(np.float64) + 1.0
    dinv = (1.0 / np.sqrt(deg)).astype(np.float32)
    order = np.argsort(deg, kind="stable")
    core_of = np.empty(N, np.int64)
    pos_of = np.empty(N, np.int64)
    core_of[order] = np.arange(N) % NCORES
    pos_of[order] = np.arange(N) // NCORES

    pb = np.array(p[1:])                       # chunk upper pos bounds
    def tid_cp(c, pos):
        k = np.searchsorted(pb - 1, pos)       # chunk of pos
        k = np.minimum(k, NCHUNK - 1)
        Lk = np.array(L)[k]
        return tbl_base[k] + c * Lk + (pos - np.array(p[:NCHUNK])[k])

    tid = tid_cp(core_of, pos_of)              # node -> table row
    zid = int(tbl_base[NCHUNK - 1] + (NCORES - 1) * L[NCHUNK - 1]
              + (SR - p[NCHUNK - 1] - 1))
    assert zid == TBL - 1

    es = np.concatenate([src, np.arange(N)])   # + self loops
    ed = np.concatenate([dst, np.arange(N)])
    sid_all = tid[es]
    ec = core_of[ed]
    ep = pos_of[ed]

    counts = np.zeros((NCORES, NTILES * 128), np.int64)
    np.add.at(counts, (ec, ep), 1)
    nb = counts.reshape(NCORES, NTILES, 128).max(axis=(0, 2))  # [NTILES]

    # greedy call grouping over the permuted tile order
    order_t, _ = _tile_order()
    calls = []                                 # (tile_list, nidx)
    cur, acc = [], 0
    for t in order_t:
        w = int(128 * nb[t])
        if acc and acc + w > CAP:
            calls.append((cur, acc))
            cur, acc = [], 0
        cur.append(t)
        acc += w
    calls.append((cur, acc))

    Sl = int((128 * nb).sum())
    tile_off = np.concatenate([[0], np.cumsum(128 * nb)])

    idx_wrapped = []
    dinv_t, dinv2_t, invd_row = [], [], []
    shard_nodes = []
    for c in range(NCORES):
        sel = ec == c
        pos = ep[sel]
        s = sid_all[sel]
        o = np.argsort(pos, kind="stable")
        pos, s = pos[o], s[o]
        cnt = np.bincount(pos, minlength=NTILES * 128)
        starts = np.concatenate([[0], np.cumsum(cnt)[:-1]])
        r = np.arange(len(pos)) - starts[pos]
        t_of = pos // 128
        jj = pos % 128
        flat = tile_off[t_of] + jj * nb[t_of] + r
        slots = np.full(Sl, zid, np.int64)
        slots[flat] = s
        stream = []
        for (tlist, nidx) in calls:
            for t in tlist:
                stream.append(slots[tile_off[t]:tile_off[t] + 128 * nb[t]])
            stream.append(np.full(128, zid, np.int64))  # positive tail pad
        arr = np.concatenate(stream)
        idx16 = (arr - BASE).astype(np.int16)
        w16 = idx16.reshape(-1, 16).T
        idx_wrapped.append(np.tile(w16, (8, 1)).copy())

        nodes = order[c::NCORES]               # pos-ordered own nodes
        shard_nodes.append(nodes)
        dv = np.ones(NTILES * 128, np.float32)
        dv[:SHARD] = dinv[nodes]
        dinv_t.append(dv.reshape(NTILES, 128).T.copy())
        dinv2_t.append((dv * dv).reshape(NTILES, 128).T.copy())
        invd_row.append((1.0 / dv)[None, :].copy())
    return (dinv, tid, nb, calls, idx_wrapped, dinv_t, dinv2_t, invd_row,
            shard_nodes, p, tbl_base, zid)


def _build(nb, calls, p, tbl_base):
    import os
    import concourse.bass as bass
    import concourse.tile as tile
    from concourse import bacc, mybir
    NO_AG = bool(int(os.environ.get("KERNEL_NO_AG", "0")))
    NLAYERS = int(os.environ.get("KERNEL_NLAYERS", "3"))

    f32 = mybir.dt.float32
    bf16 = mybir.dt.float16
    i16 = mybir.dt.int16
    AF = mybir.ActivationFunctionType
    nc = bacc.Bacc("TRN2", num_devices=NCORES, debug=False,
                   num_swdge_queues=4, dynamic_dma_scratch_size=32768)

    SC = sum((nidx + 128) // 16 for (_, nidx) in calls)
    GMAX = max(nidx for (_, nidx) in calls) + 128
    t0_in = nc.dram_tensor("t0", [TBL, D], bf16, kind="ExternalInput")
    idx_in = nc.dram_tensor("idx", [128, SC], i16, kind="ExternalInput")
    w1_in = nc.dram_tensor("w1", [128, D], f32, kind="ExternalInput")
    w2_in = nc.dram_tensor("w2", [128, D], f32, kind="ExternalInput")
    w3_in = nc.dram_tensor("w3", [128, D_OUT_PAD], f32, kind="ExternalInput")
    b1_in = nc.dram_tensor("b1r", [1, D], f32, kind="ExternalInput")
    b2_in = nc.dram_tensor("b2r", [1, D], f32, kind="ExternalInput")
    b3_in = nc.dram_tensor("b3r", [1, D_OUT_PAD], f32, kind="ExternalInput")
    invd_in = nc.dram_tensor("invd", [1, NTILES * 128], f32,
                             kind="ExternalInput")
    dinv_in = nc.dram_tensor("dinvt", [128, NTILES], f32, kind="ExternalInput")
    dinv2_in = nc.dram_tensor("dinv2t", [128, NTILES], f32,
                              kind="ExternalInput")
    y_out = nc.dram_tensor("y", [SHARD, D_OUT], f32, kind="ExternalOutput")

    with tile.TileContext(nc) as tc:
        with tc.tile_pool(name="cst", bufs=1) as cst, \
             tc.tile_pool(name="gp", bufs=4) as gp, \
             tc.tile_pool(name="wrk", bufs=4) as wrk, \
             tc.tile_pool(name="ps", bufs=4, space="PSUM") as ps, \
             tc.tile_pool(name="dram", bufs=1, space="DRAM") as dram:

            idx_sb = cst.tile([128, SC], i16)
            nc.sync.dma_start(idx_sb[:], idx_in[:, :])
            w1s = cst.tile([128, D], f32)
            nc.sync.dma_start(w1s[:], w1_in[:, :])
            w2s = cst.tile([128, D], f32)
            nc.sync.dma_start(w2s[:], w2_in[:, :])
            w3s = cst.tile([128, D_OUT_PAD], f32)
            nc.sync.dma_start(w3s[:], w3_in[:, :])
            b1s = cst.tile([1, D], f32)
            nc.sync.dma_start(b1s[:], b1_in[:, :])
            b2s = cst.tile([1, D], f32)
            nc.sync.dma_start(b2s[:], b2_in[:, :])
            b3s = cst.tile([1, D_OUT_PAD], f32)
            nc.sync.dma_start(b3s[:], b3_in[:, :])
            invd_sb = cst.tile([1, NTILES * 128], f32)
            nc.sync.dma_start(invd_sb[:], invd_in[:, :])
            dinv_sb = cst.tile([128, NTILES], f32)
            nc.sync.dma_start(dinv_sb[:], dinv_in[:, :])
            dinv2_sb = cst.tile([128, NTILES], f32)
            nc.sync.dma_start(dinv2_sb[:], dinv2_in[:, :])
            zrow = cst.tile([128, D], bf16)
            nc.vector.memset(zrow[:], 0.0)

            sh1 = dram.tile([SR, D], bf16, tag="sh1")
            sh2 = dram.tile([SR, D], bf16, tag="sh2")
            # chunked tables: contiguous Shared tensors, one AG writer each
            tchunks = []
            for ln in (1, 2):
                cs = [nc.dram_tensor(f"tb{ln}c{k}",
                                     [NCORES * (p[k + 1] - p[k]), D], bf16,
                                     addr_space="Shared")
                      for k in range(NCHUNK)]
                a0 = nc.lookup_mls(cs[0]).memorylocations[0].addr
                for k in range(1, NCHUNK):
                    ak = nc.lookup_mls(cs[k]).memorylocations[0].addr
                    exp = a0 + tbl_base[k] * D * 2
                    assert ak == exp, (ln, k, ak, exp)
                tchunks.append(cs)

            _, chunk_sets = _tile_order()
            chunk_of_tile = {}
            for ci, cs in enumerate(chunk_sets):
                for t in cs:
                    chunk_of_tile[t] = ci
            rg = [list(range(NCORES))]

            def emit_ag(sh, chunks, ck):
                if NO_AG:
                    return
                nc.gpsimd.collective_compute(
                    "AllGather", mybir.AluOpType.bypass,
                    replica_groups=rg,
                    ins=[sh[p[ck]:p[ck + 1], :].opt()],
                    outs=[chunks[ck][:, :].opt()])

            for l in range(NLAYERS):
                fo = D if l < 2 else D_OUT_PAD
                W = (w1s, w2s, w3s)[l]
                br = (b1s, b2s, b3s)[l]
                src = (t0_in[BASE:, :] if l == 0
                       else tchunks[l - 1][BASE_CHUNK][:, :])
                sh = (sh1, sh2, None)[l]
                ch_next = (tchunks[0], tchunks[1], None)[l]
                pend_ag = []
                col = 0
                done = [0] * NCHUNK   # tiles emitted per chunk
                for k, (tlist, nidx) in enumerate(calls):
                    nid = nidx + 128
                    gout = gp.tile([128, 1, GMAX], bf16, tag="g")
                    nc.gpsimd.dma_gather(
                        out_ap=gout[:, :, :nid],
                        in_ap=src,
                        idxs_ap=idx_sb[:, col:col + nid // 16],
                        num_idxs=nid, num_idxs_reg=nid, elem_size=D,
                        transpose=True, single_packet=False, queue_num=k % 4)
                    col += nid // 16
                    while (pend_ag and pend_ag[0][0] <= k
                           and k < len(calls) - 1):
                        emit_ag(sh, ch_next, pend_ag.pop(0)[1])
                    off = 0
                    for t in tlist:
                        nbt = int(nb[t])
                        pt = min(128, SHARD - t * 128)
                        agg = wrk.tile([128, 128], f32, tag="agg")
                        v = gout[:, 0, off:off + 128 * nbt].rearrange(
                            "q (j b) -> q j b", b=nbt)
                        nc.vector.tensor_reduce(agg[:], v,
                                                axis=mybir.AxisListType.X,
                                                op=mybir.AluOpType.add)
                        off += 128 * nbt
                        pst = ps.tile([128, fo], f32, tag="ps")
                        nc.tensor.matmul(pst[:], lhsT=agg[:], rhs=W[:],
                                         start=True, stop=False)
                        nc.tensor.matmul(
                            pst[:],
                            lhsT=invd_sb[0:1, t * 128:(t + 1) * 128],
                            rhs=br[0:1, :], start=False, stop=True)
                        if l < 2:
                            hsb = wrk.tile([128, D], bf16, tag="hsb")
                            nc.scalar.activation(hsb[:], pst[:], AF.Relu,
                                                 scale=dinv2_sb[:, t:t + 1])
                            nc.sync.dma_start(sh[t * 128:t * 128 + pt, :],
                                              hsb[:pt, :])
                            if t == NTILES - 1:
                                nc.sync.dma_start(sh[SHARD:SR, :],
                                                  zrow[0:SR - SHARD, :])
                            ck = chunk_of_tile[t]
                            done[ck] += 1
                            if done[ck] == CHUNK_TILES[ck]:
                                pend_ag.append((k + 2, ck))
                        else:
                            zt = wrk.tile([128, D_OUT_PAD], f32, tag="zt")
                            nc.scalar.activation(zt[:], pst[:], AF.Copy,
                                                 scale=dinv_sb[:, t:t + 1])
                            mx = wrk.tile([128, 1], f32, tag="mx")
                            nc.vector.tensor_reduce(mx[:], zt[:, :D_OUT],
                                                    axis=mybir.AxisListType.X,
                                                    op=mybir.AluOpType.max)
                            nmx = wrk.tile([128, 1], f32, tag="nmx")
                            nc.vector.tensor_scalar_mul(nmx[:], mx[:], -1.0)
                            ex = wrk.tile([128, D_OUT], f32, tag="ex")
                            se = wrk.tile([128, 1], f32, tag="se")
                            nc.scalar.activation(ex[:], zt[:, :D_OUT],
                                                 AF.Exp, bias=nmx[:, 0:1],
                                                 accum_out=se[:, 0:1])
                            lse = wrk.tile([128, 1], f32, tag="lse")
                            nc.scalar.activation(lse[:], se[:], AF.Ln)
                            ot = wrk.tile([128, D_OUT], f32, tag="ot")
                            nc.vector.tensor_scalar(
                                ot[:], zt[:, :D_OUT],
                                scalar1=mx[:, 0:1], scalar2=lse[:, 0:1],
                                op0=mybir.AluOpType.subtract,
                                op1=mybir.AluOpType.subtract)
                            nc.sync.dma_start(y_out[t * 128:t * 128 + pt, :],
                                              ot[:pt, :])
                # layer-end flush: remaining chunk AGs in completion order
                # (BASE chunk's tiles were processed last, so it flushes last)
                for (_, ck) in pend_ag:
                    emit_ag(sh, ch_next, ck)

    nc.compile()
    return nc


def prepare(x, src, dst, W1, b1, W2, b2, W3, b3,
            g1, be1, m1, v1, g2, be2, m2, v2):
    bf = np.float16
    x = np.asarray(x, np.float32)
    src = np.asarray(src, np.int64)
    dst = np.asarray(dst, np.int64)
    (dinv, tid, nb, calls, idx_wrapped, dinv_t, dinv2_t, invd_row,
     shard_nodes, p, tbl_base, zid) = _preprocess(src, dst)
    nc = _build(nb, calls, p, tbl_base)

    s1 = np.asarray(g1, np.float32) / np.sqrt(np.asarray(v1, np.float32)
                                              + BN_EPS)
    s2 = np.asarray(g2, np.float32) / np.sqrt(np.asarray(v2, np.float32)
                                              + BN_EPS)
    w1p = np.asarray(W1, np.float32) * s1[None, :]
    w2p = np.asarray(W2, np.float32) * s2[None, :]
    b1p = ((np.asarray(b1, np.float32) - np.asarray(m1, np.float32)) * s1
           + np.asarray(be1, np.float32))[None, :]
    b2p = ((np.asarray(b2, np.float32) - np.asarray(m2, np.float32)) * s2
           + np.asarray(be2, np.float32))[None, :]
    w3p = np.zeros((128, D_OUT_PAD), np.float32)
    w3p[:, :D_OUT] = np.asarray(W3, np.float32)
    b3p = np.zeros((1, D_OUT_PAD), np.float32)
    b3p[0, :D_OUT] = np.asarray(b3, np.float32)

    t0 = np.zeros((TBL, D), np.float32)
    t0[tid] = x * dinv[:, None]
    t0 = t0.astype(bf)

    in_maps = []
    for c in range(NCORES):
        in_maps.append({
            "t0": t0, "idx": idx_wrapped[c],
            "w1": w1p, "w2": w2p, "w3": w3p,
            "b1r": b1p, "b2r": b2p, "b3r": b3p,
            "invd": invd_row[c],
            "dinvt": dinv_t[c], "dinv2t": dinv2_t[c],
        })
    return nc, in_maps, shard_nodes


def kernel(**inputs):
    from concourse.bass_utils import run_bass_kernel_spmd

    nc, in_maps, shard_nodes = prepare(**inputs)
    res = run_bass_kernel_spmd(nc, in_maps, core_ids=list(range(NCORES)))
    out = np.zeros((N, D_OUT), np.float32)
    for c in range(NCORES):
        out[shard_nodes[c][:SHARD]] = res.results[c]["y"]
    return out


# revision 36
# speedup vs baseline: 1.7285x; 1.4269x over previous
"""3-layer GCN (GCNConv+BN+ReLU x2, GCNConv+log_softmax) on 8 trn2 NeuronCores.

Strategy (v2): aggregate in input space, transform after. Nodes are
in-degree-sorted and dealt round-robin to 8 cores. Tables T_l hold
h_l(n)*dinv[n] in bf16, node-major (T0 = x*dinv precomputed on host, so
layer 1 needs no collective). Per layer, each core runs a few BIG
transpose-mode dma_gather calls (feature-major output, j-major slot packing
per 128-dst tile), a DVE tensor_reduce per tile for the segment sum, one
PE matmul agg'@W' (+ rank-1 bias matmul via 1/dinv row), and one fused ACT
(relu, scale=dinv^2) producing the next table row, written to the shard.
AllGathers are chunked (4 per layer) and dispatched two gather-calls late
so the CC engine overlaps them with remaining gathers; only the small last
chunk is exposed at the layer boundary. Gather indices are int16 signed
offsets around a mid-table BASE (HW sign-extends); every call is tail-padded
with one block of positive zero-row indices so the trailing-negative drop
rule never bites. The runtime is descriptor-generation-bound on GPSIMD, so
everything else is engineered to hide under it.
"""
import numpy as np

N = 50000
E = 800000
D = 128
D_OUT = 40
D_OUT_PAD = 64
BN_EPS = 1e-5
NCORES = 8
SHARD = N // NCORES              # 6250
SR = 6256                        # shard rows (padded)
TBL = SR * NCORES                # 50048
NTILES = (SHARD + 127) // 128    # 49
BASE = 32768
CAP = 3072                       # max real idxs per gather call
# AllGather chunking (in tiles). The chunk whose table range contains BASE
# (rows 32768..) must be dispatched LAST so the gather's dep on it implies
# all earlier chunks completed (CC queue is in-order). Chunk 2 starts at
# table row 8*4096 = 32768 by construction.
CHUNK_TILES = (16, 16, 2, 11, 4)
NCHUNK = len(CHUNK_TILES)
BASE_CHUNK = 2
# tile processing order: BASE_CHUNK's tiles go last, so its AG is the only
# one exposed at the layer boundary (all other chunk AGs overlap gathers)
def _tile_order():
    b = np.cumsum((0,) + CHUNK_TILES)
    chunks = [list(range(b[i], b[i + 1])) for i in range(NCHUNK)]
    order = []
    for i in range(NCHUNK):
        if i != BASE_CHUNK:
            order += chunks[i]
    order += chunks[BASE_CHUNK]
    return order, [set(c) for c in chunks]


def _chunk_layout():
    b = np.cumsum((0,) + CHUNK_TILES)          # tile bounds, b[-1] == 49
    p = [int(min(x * 128, SR)) for x in b]
    p[-1] = SR                                  # last chunk includes pad rows
    L = [p[i + 1] - p[i] for i in range(NCHUNK)]
    tbl_base = np.concatenate([[0], np.cumsum([NCORES * x for x in L])])
    assert tbl_base[BASE_CHUNK] == BASE
    return p, L, tbl_base


def _preprocess(src, dst):
    import os
    global CAP
    CAP = int(os.environ.get("KERNEL_CAP", CAP))
    p, L, tbl_base = _chunk_layout()
    deg = np.bincount(dst, minlength=N).astype(np.float64) + 1.0
    dinv = (1.0 / np.sqrt(deg)).astype(np.float32)
    order = np.argsort(deg, kind="stable")
    core_of = np.empty(N, np.int64)
    pos_of = np.empty(N, np.int64)
    core_of[order] = np.arange(N) % NCORES
    pos_of[order] = np.arange(N) // NCORES

    pb = np.array(p[1:])                       # chunk upper pos bounds
    def tid_cp(c, pos):
        k = np.searchsorted(pb - 1, pos)       # chunk of pos
        k = np.minimum(k, NCHUNK - 1)
        Lk = np.array(L)[k]
        return tbl_base[k] + c * Lk + (pos - np.array(p[:NCHUNK])[k])

    tid = tid_cp(core_of, pos_of)              # node -> table row
    zid = int(tbl_base[NCHUNK - 1] + (NCORES - 1) * L[NCHUNK - 1]
              + (SR - p[NCHUNK - 1] - 1))
    assert zid == TBL - 1

    es = np.concatenate([src, np.arange(N)])   # + self loops
    ed = np.concatenate([dst, np.arange(N)])
    sid_all = tid[es]
    ec = core_of[ed]
    ep = pos_of[ed]

    counts = np.zeros((NCORES, NTILES * 128), np.int64)
    np.add.at(counts, (ec, ep), 1)
    nb = counts.reshape(NCORES, NTILES, 128).max(axis=(0, 2))  # [NTILES]

    # greedy call grouping over the permuted tile order
    order_t, _ = _tile_order()
    calls = []                                 # (tile_list, nidx)
    cur, acc = [], 0
    for t in order_t:
        w = int(128 * nb[t])
        if acc and acc + w > CAP:
            calls.append((cur, acc))
            cur, acc = [], 0
        cur.append(t)
        acc += w
    calls.append((cur, acc))

    Sl = int((128 * nb).sum())
    tile_off = np.concatenate([[0], np.cumsum(128 * nb)])

    idx_wrapped = []
    dinv_t, dinv2_t, invd_row = [], [], []
    shard_nodes = []
    for c in range(NCORES):
        sel = ec == c
        pos = ep[sel]
        s = sid_all[sel]
        o = np.argsort(pos, kind="stable")
        pos, s = pos[o], s[o]
        cnt = np.bincount(pos, minlength=NTILES * 128)
        starts = np.concatenate([[0], np.cumsum(cnt)[:-1]])
        r = np.arange(len(pos)) - starts[pos]
        t_of = pos // 128
        jj = pos % 128
        flat = tile_off[t_of] + jj * nb[t_of] + r
        slots = np.full(Sl, zid, np.int64)
        slots[flat] = s
        stream = []
        for (tlist, nidx) in calls:
            for t in tlist:
                stream.append(slots[tile_off[t]:tile_off[t] + 128 * nb[t]])
            stream.append(np.full(128, zid, np.int64))  # positive tail pad
        arr = np.concatenate(stream)
        idx16 = (arr - BASE).astype(np.int16)
        w16 = idx16.reshape(-1, 16).T
        idx_wrapped.append(np.tile(w16, (8, 1)).copy())

        nodes = order[c::NCORES]               # pos-ordered own nodes
        shard_nodes.append(nodes)
        dv = np.ones(NTILES * 128, np.float32)
        dv[:SHARD] = dinv[nodes]
        dinv_t.append(dv.reshape(NTILES, 128).T.copy())
        dinv2_t.append((dv * dv).reshape(NTILES, 128).T.copy())
        invd_row.append((1.0 / dv)[None, :].copy())
    return (dinv, tid, nb, calls, idx_wrapped, dinv_t, dinv2_t, invd_row,
            shard_nodes, p, tbl_base, zid)


def _build(nb, calls, p, tbl_base):
    import os
    import concourse.bass as bass
    import concourse.tile as tile
    from concourse import bacc, mybir
    NO_AG = bool(int(os.environ.get("KERNEL_NO_AG", "0")))
    NLAYERS = int(os.environ.get("KERNEL_NLAYERS", "3"))
    DBG = bool(int(os.environ.get("KERNEL_DBG", "0")))
    NQ = int(os.environ.get("KERNEL_NQ", "4"))

    f32 = mybir.dt.float32
    bf16 = mybir.dt.float16
    i16 = mybir.dt.int16
    AF = mybir.ActivationFunctionType
    nc = bacc.Bacc("TRN2", num_devices=NCORES, debug=False,
                   num_swdge_queues=4, dynamic_dma_scratch_size=32768)

    SC = sum((nidx + 128) // 16 for (_, nidx) in calls)
    GMAX = max(nidx for (_, nidx) in calls) + 128
    t0_in = nc.dram_tensor("t0", [TBL, D], bf16, kind="ExternalInput")
    idx_in = nc.dram_tensor("idx", [128, SC], i16, kind="ExternalInput")
    w1_in = nc.dram_tensor("w1", [128, D], f32, kind="ExternalInput")
    w2_in = nc.dram_tensor("w2", [128, D], f32, kind="ExternalInput")
    w3_in = nc.dram_tensor("w3", [128, D_OUT_PAD], f32, kind="ExternalInput")
    b1_in = nc.dram_tensor("b1r", [1, D], f32, kind="ExternalInput")
    b2_in = nc.dram_tensor("b2r", [1, D], f32, kind="ExternalInput")
    b3_in = nc.dram_tensor("b3r", [1, D_OUT_PAD], f32, kind="ExternalInput")
    invd_in = nc.dram_tensor("invd", [1, NTILES * 128], f32,
                             kind="ExternalInput")
    dinv_in = nc.dram_tensor("dinvt", [128, NTILES], f32, kind="ExternalInput")
    dinv2_in = nc.dram_tensor("dinv2t", [128, NTILES], f32,
                              kind="ExternalInput")
    y_out = nc.dram_tensor("y", [SHARD, D_OUT], f32, kind="ExternalOutput")
    if DBG:
        dbg_sh = nc.dram_tensor("dbgsh", [SR, D], bf16, kind="ExternalOutput")
        dbg_tb = nc.dram_tensor("dbgtb", [TBL, D], bf16, kind="ExternalOutput")

    with tile.TileContext(nc) as tc:
        with tc.tile_pool(name="cst", bufs=1) as cst, \
             tc.tile_pool(name="gp", bufs=4) as gp, \
             tc.tile_pool(name="wrk", bufs=4) as wrk, \
             tc.tile_pool(name="ps", bufs=4, space="PSUM") as ps, \
             tc.tile_pool(name="dram", bufs=1, space="DRAM") as dram:

            idx_sb = cst.tile([128, SC], i16)
            nc.sync.dma_start(idx_sb[:], idx_in[:, :])
            w1s = cst.tile([128, D], f32)
            nc.sync.dma_start(w1s[:], w1_in[:, :])
            w2s = cst.tile([128, D], f32)
            nc.sync.dma_start(w2s[:], w2_in[:, :])
            w3s = cst.tile([128, D_OUT_PAD], f32)
            nc.sync.dma_start(w3s[:], w3_in[:, :])
            b1s = cst.tile([1, D], f32)
            nc.sync.dma_start(b1s[:], b1_in[:, :])
            b2s = cst.tile([1, D], f32)
            nc.sync.dma_start(b2s[:], b2_in[:, :])
            b3s = cst.tile([1, D_OUT_PAD], f32)
            nc.sync.dma_start(b3s[:], b3_in[:, :])
            invd_sb = cst.tile([1, NTILES * 128], f32)
            nc.sync.dma_start(invd_sb[:], invd_in[:, :])
            dinv_sb = cst.tile([128, NTILES], f32)
            nc.sync.dma_start(dinv_sb[:], dinv_in[:, :])
            dinv2_sb = cst.tile([128, NTILES], f32)
            nc.sync.dma_start(dinv2_sb[:], dinv2_in[:, :])
            zrow = cst.tile([128, D], bf16)
            nc.vector.memset(zrow[:], 0.0)

            sh1 = dram.tile([SR, D], bf16, tag="sh1")
            sh2 = dram.tile([SR, D], bf16, tag="sh2")
            # chunked tables: contiguous Shared tensors, one AG writer each
            tchunks = []
            for ln in (1, 2):
                cs = [nc.dram_tensor(f"tb{ln}c{k}",
                                     [NCORES * (p[k + 1] - p[k]), D], bf16,
                                     addr_space="Shared")
                      for k in range(NCHUNK)]
                a0 = nc.lookup_mls(cs[0]).memorylocations[0].addr
                for k in range(1, NCHUNK):
                    ak = nc.lookup_mls(cs[k]).memorylocations[0].addr
                    exp = a0 + tbl_base[k] * D * 2
                    assert ak == exp, (ln, k, ak, exp)
                tchunks.append(cs)

            _, chunk_sets = _tile_order()
            chunk_of_tile = {}
            for ci, cs in enumerate(chunk_sets):
                for t in cs:
                    chunk_of_tile[t] = ci
            rg = [list(range(NCORES))]

            def emit_ag(sh, chunks, ck):
                if NO_AG:
                    return
                nc.gpsimd.collective_compute(
                    "AllGather", mybir.AluOpType.bypass,
                    replica_groups=rg,
                    ins=[sh[p[ck]:p[ck + 1], :].opt()],
                    outs=[chunks[ck][:, :].opt()])

            for l in range(NLAYERS):
                fo = D if l < 2 else D_OUT_PAD
                W = (w1s, w2s, w3s)[l]
                br = (b1s, b2s, b3s)[l]
                src = (t0_in[BASE:, :] if l == 0
                       else tchunks[l - 1][BASE_CHUNK][:, :])
                sh = (sh1, sh2, None)[l]
                ch_next = (tchunks[0], tchunks[1], None)[l]
                pend_ag = []
                col = 0
                done = [0] * NCHUNK   # tiles emitted per chunk
                for k, (tlist, nidx) in enumerate(calls):
                    nid = nidx + 128
                    gout = gp.tile([128, 1, GMAX], bf16, tag="g")
                    nc.gpsimd.dma_gather(
                        out_ap=gout[:, :, :nid],
                        in_ap=src,
                        idxs_ap=idx_sb[:, col:col + nid // 16],
                        num_idxs=nid, num_idxs_reg=nid, elem_size=D,
                        transpose=True, single_packet=False,
                        queue_num=k % NQ)
                    col += nid // 16
                    while (pend_ag and pend_ag[0][0] <= k
                           and k < len(calls) - 1):
                        emit_ag(sh, ch_next, pend_ag.pop(0)[1])
                    off = 0
                    for t in tlist:
                        nbt = int(nb[t])
                        pt = min(128, SHARD - t * 128)
                        agg = wrk.tile([128, 128], f32, tag="agg")
                        v = gout[:, 0, off:off + 128 * nbt].rearrange(
                            "q (j b) -> q j b", b=nbt)
                        nc.vector.tensor_reduce(agg[:], v,
                                                axis=mybir.AxisListType.X,
                                                op=mybir.AluOpType.add)
                        off += 128 * nbt
                        pst = ps.tile([128, fo], f32, tag="ps")
                        nc.tensor.matmul(pst[:], lhsT=agg[:], rhs=W[:],
                                         start=True, stop=False)
                        nc.tensor.matmul(
                            pst[:],
                            lhsT=invd_sb[0:1, t * 128:(t + 1) * 128],
                            rhs=br[0:1, :], start=False, stop=True)
                        if l < 2:
                            hsb = wrk.tile([128, D], bf16, tag="hsb")
                            nc.scalar.activation(hsb[:], pst[:], AF.Relu,
                                                 scale=dinv2_sb[:, t:t + 1])
                            nc.sync.dma_start(sh[t * 128:t * 128 + pt, :],
                                              hsb[:pt, :])
                            if t == NTILES - 1:
                                nc.sync.dma_start(sh[SHARD:SR, :],
                                                  zrow[0:SR - SHARD, :])
                            ck = chunk_of_tile[t]
                            done[ck] += 1
                            if done[ck] == CHUNK_TILES[ck]:
                                pend_ag.append((k + 2, ck))
                        else:
                            zt = wrk.tile([128, D_OUT_PAD], f32, tag="zt")
                            nc.scalar.activation(zt[:], pst[:], AF.Copy,
                                                 scale=dinv_sb[:, t:t + 1])
                            mx = wrk.tile([128, 1], f32, tag="mx")
                            nc.vector.tensor_reduce(mx[:], zt[:, :D_OUT],
                                                    axis=mybir.AxisListType.X,
                                                    op=mybir.AluOpType.max)
                            nmx = wrk.tile([128, 1], f32, tag="nmx")
                            nc.vector.tensor_scalar_mul(nmx[:], mx[:], -1.0)
                            ex = wrk.tile([128, D_OUT], f32, tag="ex")
                            se = wrk.tile([128, 1], f32, tag="se")
                            nc.scalar.activation(ex[:], zt[:, :D_OUT],
                                                 AF.Exp, bias=nmx[:, 0:1],
                                                 accum_out=se[:, 0:1])
                            lse = wrk.tile([128, 1], f32, tag="lse")
                            nc.scalar.activation(lse[:], se[:], AF.Ln)
                            ot = wrk.tile([128, D_OUT], f32, tag="ot")
                            nc.vector.tensor_scalar(
                                ot[:], zt[:, :D_OUT],
                                scalar1=mx[:, 0:1], scalar2=lse[:, 0:1],
                                op0=mybir.AluOpType.subtract,
                                op1=mybir.AluOpType.subtract)
                            nc.sync.dma_start(y_out[t * 128:t * 128 + pt, :],
                                              ot[:pt, :])
                # layer-end flush: remaining chunk AGs in completion order
                # (BASE chunk's tiles were processed last, so it flushes last)
                for (_, ck) in pend_ag:
                    emit_ag(sh, ch_next, ck)

            if DBG:
                nc.sync.dma_start(dbg_sh[:, :], sh1[:, :])
                for ck in range(NCHUNK):
                    nc.sync.dma_start(
                        dbg_tb[tbl_base[ck]:tbl_base[ck + 1], :],
                        tchunks[0][ck][:, :])

    nc.compile()
    return nc


def prepare(x, src, dst, W1, b1, W2, b2, W3, b3,
            g1, be1, m1, v1, g2, be2, m2, v2):
    bf = np.float16
    x = np.asarray(x, np.float32)
    src = np.asarray(src, np.int64)
    dst = np.asarray(dst, np.int64)
    (dinv, tid, nb, calls, idx_wrapped, dinv_t, dinv2_t, invd_row,
     shard_nodes, p, tbl_base, zid) = _preprocess(src, dst)
    nc = _build(nb, calls, p, tbl_base)

    s1 = np.asarray(g1, np.float32) / np.sqrt(np.asarray(v1, np.float32)
                                              + BN_EPS)
    s2 = np.asarray(g2, np.float32) / np.sqrt(np.asarray(v2, np.float32)
                                              + BN_EPS)
    w1p = np.asarray(W1, np.float32) * s1[None, :]
    w2p = np.asarray(W2, np.float32) * s2[None, :]
    b1p = ((np.asarray(b1, np.float32) - np.asarray(m1, np.float32)) * s1
           + np.asarray(be1, np.float32))[None, :]
    b2p = ((np.asarray(b2, np.float32) - np.asarray(m2, np.float32)) * s2
           + np.asarray(be2, np.float32))[None, :]
    w3p = np.zeros((128, D_OUT_PAD), np.float32)
    w3p[:, :D_OUT] = np.asarray(W3, np.float32)
    b3p = np.zeros((1, D_OUT_PAD), np.float32)
    b3p[0, :D_OUT] = np.asarray(b3, np.float32)

    t0 = np.zeros((TBL, D), np.float32)
    t0[tid] = x * dinv[:, None]
    t0 = t0.astype(bf)

    in_maps = []
    for c in range(NCORES):
        in_maps.append({
            "t0": t0, "idx": idx_wrapped[c],
            "w1": w1p, "w2": w2p, "w3": w3p,
            "b1r": b1p, "b2r": b2p, "b3r": b3p,
            "invd": invd_row[c],
            "dinvt": dinv_t[c], "dinv2t": dinv2_t[c],
        })
    return nc, in_maps, shard_nodes


def kernel(**inputs):
    from concourse.bass_utils import run_bass_kernel_spmd

    nc, in_maps, shard_nodes = prepare(**inputs)
    res = run_bass_kernel_spmd(nc, in_maps, core_ids=list(range(NCORES)))
    out = np.zeros((N, D_OUT), np.float32)
    for c in range(NCORES):
        out[shard_nodes[c][:SHARD]] = res.results[c]["y"]
    return out


# revision 37
# speedup vs baseline: 2.0695x; 1.1973x over previous
"""3-layer GCN (GCNConv+BN+ReLU x2, GCNConv+log_softmax) on 8 trn2 NeuronCores.

Strategy (v2): aggregate in input space, transform after. Nodes are
in-degree-sorted and dealt round-robin to 8 cores. Tables T_l hold
h_l(n)*dinv[n] in bf16, node-major (T0 = x*dinv precomputed on host, so
layer 1 needs no collective). Per layer, each core runs a few BIG
transpose-mode dma_gather calls (feature-major output, j-major slot packing
per 128-dst tile), a DVE tensor_reduce per tile for the segment sum, one
PE matmul agg'@W' (+ rank-1 bias matmul via 1/dinv row), and one fused ACT
(relu, scale=dinv^2) producing the next table row, written to the shard.
AllGathers are chunked (4 per layer) and dispatched two gather-calls late
so the CC engine overlaps them with remaining gathers; only the small last
chunk is exposed at the layer boundary. Gather indices are int16 signed
offsets around a mid-table BASE (HW sign-extends); every call is tail-padded
with one block of positive zero-row indices so the trailing-negative drop
rule never bites. The runtime is descriptor-generation-bound on GPSIMD, so
everything else is engineered to hide under it.
"""
import numpy as np

N = 50000
E = 800000
D = 128
D_OUT = 40
D_OUT_PAD = 64
BN_EPS = 1e-5
NCORES = 8
SHARD = N // NCORES              # 6250
SR = 6256                        # shard rows (padded)
TBL = SR * NCORES                # 50048
NTILES = (SHARD + 127) // 128    # 49
BASE = 32768
CAP = 3072                       # max real idxs per gather call
# AllGather chunking (in tiles). The chunk whose table range contains BASE
# (rows 32768..) must be dispatched LAST so the gather's dep on it implies
# all earlier chunks completed (CC queue is in-order). Chunk 2 starts at
# table row 8*4096 = 32768 by construction.
CHUNK_TILES = (16, 16, 2, 11, 4)
NCHUNK = len(CHUNK_TILES)
BASE_CHUNK = 2
# tile processing order: BASE_CHUNK's tiles go last, so its AG is the only
# one exposed at the layer boundary (all other chunk AGs overlap gathers)
def _tile_order():
    b = np.cumsum((0,) + CHUNK_TILES)
    chunks = [list(range(b[i], b[i + 1])) for i in range(NCHUNK)]
    order = []
    for i in range(NCHUNK):
        if i != BASE_CHUNK:
            order += chunks[i]
    order += chunks[BASE_CHUNK]
    return order, [set(c) for c in chunks]


def _chunk_layout():
    b = np.cumsum((0,) + CHUNK_TILES)          # tile bounds, b[-1] == 49
    p = [int(min(x * 128, SR)) for x in b]
    p[-1] = SR                                  # last chunk includes pad rows
    L = [p[i + 1] - p[i] for i in range(NCHUNK)]
    tbl_base = np.concatenate([[0], np.cumsum([NCORES * x for x in L])])
    assert tbl_base[BASE_CHUNK] == BASE
    return p, L, tbl_base


def _preprocess(src, dst):
    import os
    global CAP
    CAP = int(os.environ.get("KERNEL_CAP", CAP))
    p, L, tbl_base = _chunk_layout()
    deg = np.bincount(dst, minlength=N).astype(np.float64) + 1.0
    dinv = (1.0 / np.sqrt(deg)).astype(np.float32)
    order = np.argsort(deg, kind="stable")
    core_of = np.empty(N, np.int64)
    pos_of = np.empty(N, np.int64)
    core_of[order] = np.arange(N) % NCORES
    pos_of[order] = np.arange(N) // NCORES

    pb = np.array(p[1:])                       # chunk upper pos bounds
    def tid_cp(c, pos):
        k = np.searchsorted(pb - 1, pos)       # chunk of pos
        k = np.minimum(k, NCHUNK - 1)
        Lk = np.array(L)[k]
        return tbl_base[k] + c * Lk + (pos - np.array(p[:NCHUNK])[k])

    tid = tid_cp(core_of, pos_of)              # node -> table row
    zid = int(tbl_base[NCHUNK - 1] + (NCORES - 1) * L[NCHUNK - 1]
              + (SR - p[NCHUNK - 1] - 1))
    assert zid == TBL - 1

    es = np.concatenate([src, np.arange(N)])   # + self loops
    ed = np.concatenate([dst, np.arange(N)])
    sid_all = tid[es]
    ec = core_of[ed]
    ep = pos_of[ed]

    counts = np.zeros((NCORES, NTILES * 128), np.int64)
    np.add.at(counts, (ec, ep), 1)
    nb = counts.reshape(NCORES, NTILES, 128).max(axis=(0, 2))  # [NTILES]

    # greedy call grouping over the permuted tile order
    order_t, _ = _tile_order()
    calls = []                                 # (tile_list, nidx)
    cur, acc = [], 0
    for t in order_t:
        w = int(128 * nb[t])
        if acc and acc + w > CAP:
            calls.append((cur, acc))
            cur, acc = [], 0
        cur.append(t)
        acc += w
    calls.append((cur, acc))

    Sl = int((128 * nb).sum())
    tile_off = np.concatenate([[0], np.cumsum(128 * nb)])

    idx_wrapped = []
    dinv_t, dinv2_t, invd_row = [], [], []
    shard_nodes = []
    for c in range(NCORES):
        sel = ec == c
        pos = ep[sel]
        s = sid_all[sel]
        o = np.argsort(pos, kind="stable")
        pos, s = pos[o], s[o]
        cnt = np.bincount(pos, minlength=NTILES * 128)
        starts = np.concatenate([[0], np.cumsum(cnt)[:-1]])
        r = np.arange(len(pos)) - starts[pos]
        t_of = pos // 128
        jj = pos % 128
        flat = tile_off[t_of] + jj * nb[t_of] + r
        slots = np.full(Sl, zid, np.int64)
        slots[flat] = s
        stream = []
        for (tlist, nidx) in calls:
            for t in tlist:
                stream.append(slots[tile_off[t]:tile_off[t] + 128 * nb[t]])
            stream.append(np.full(128, zid, np.int64))  # positive tail pad
        arr = np.concatenate(stream)
        idx16 = (arr - BASE).astype(np.int16)
        w16 = idx16.reshape(-1, 16).T
        idx_wrapped.append(np.tile(w16, (8, 1)).copy())

        nodes = order[c::NCORES]               # pos-ordered own nodes
        shard_nodes.append(nodes)
        dv = np.ones(NTILES * 128, np.float32)
        dv[:SHARD] = dinv[nodes]
        dinv_t.append(dv.reshape(NTILES, 128).T.copy())
        dinv2_t.append((dv * dv).reshape(NTILES, 128).T.copy())
        invd_row.append((1.0 / dv)[None, :].copy())
    return (dinv, tid, nb, calls, idx_wrapped, dinv_t, dinv2_t, invd_row,
            shard_nodes, p, tbl_base, zid)


def _build(nb, calls, p, tbl_base):
    import os
    import concourse.bass as bass
    import concourse.tile as tile
    from concourse import bacc, mybir
    NO_AG = bool(int(os.environ.get("KERNEL_NO_AG", "0")))
    NLAYERS = int(os.environ.get("KERNEL_NLAYERS", "3"))
    DBG = bool(int(os.environ.get("KERNEL_DBG", "0")))
    NQ = int(os.environ.get("KERNEL_NQ", "4"))
    GBUFS = int(os.environ.get("KERNEL_GBUFS", "4"))

    f32 = mybir.dt.float32
    bf16 = mybir.dt.float16
    i16 = mybir.dt.int16
    AF = mybir.ActivationFunctionType
    nc = bacc.Bacc("TRN2", num_devices=NCORES, debug=False,
                   num_swdge_queues=4, dynamic_dma_scratch_size=32768)

    SC = sum((nidx + 128) // 16 for (_, nidx) in calls)
    GMAX = max(nidx for (_, nidx) in calls) + 128
    t0_in = nc.dram_tensor("t0", [TBL, D], bf16, kind="ExternalInput")
    idx_in = nc.dram_tensor("idx", [128, SC], i16, kind="ExternalInput")
    w1_in = nc.dram_tensor("w1", [128, D], f32, kind="ExternalInput")
    w2_in = nc.dram_tensor("w2", [128, D], f32, kind="ExternalInput")
    w3_in = nc.dram_tensor("w3", [128, D_OUT_PAD], f32, kind="ExternalInput")
    b1_in = nc.dram_tensor("b1r", [1, D], f32, kind="ExternalInput")
    b2_in = nc.dram_tensor("b2r", [1, D], f32, kind="ExternalInput")
    b3_in = nc.dram_tensor("b3r", [1, D_OUT_PAD], f32, kind="ExternalInput")
    invd_in = nc.dram_tensor("invd", [1, NTILES * 128], f32,
                             kind="ExternalInput")
    dinv_in = nc.dram_tensor("dinvt", [128, NTILES], f32, kind="ExternalInput")
    dinv2_in = nc.dram_tensor("dinv2t", [128, NTILES], f32,
                              kind="ExternalInput")
    y_out = nc.dram_tensor("y", [SHARD, D_OUT], f32, kind="ExternalOutput")
    if DBG:
        dbg_sh = nc.dram_tensor("dbgsh", [SR, D], bf16, kind="ExternalOutput")
        dbg_tb = nc.dram_tensor("dbgtb", [TBL, D], bf16, kind="ExternalOutput")

    with tile.TileContext(nc) as tc:
        with tc.tile_pool(name="cst", bufs=1) as cst, \
             tc.tile_pool(name="gp", bufs=GBUFS) as gp, \
             tc.tile_pool(name="wrk", bufs=4) as wrk, \
             tc.tile_pool(name="ps", bufs=4, space="PSUM") as ps, \
             tc.tile_pool(name="dram", bufs=1, space="DRAM") as dram:

            idx_sb = cst.tile([128, SC], i16)
            nc.sync.dma_start(idx_sb[:], idx_in[:, :])
            w1s = cst.tile([128, D], f32)
            nc.sync.dma_start(w1s[:], w1_in[:, :])
            w2s = cst.tile([128, D], f32)
            nc.sync.dma_start(w2s[:], w2_in[:, :])
            w3s = cst.tile([128, D_OUT_PAD], f32)
            nc.sync.dma_start(w3s[:], w3_in[:, :])
            b1s = cst.tile([1, D], f32)
            nc.sync.dma_start(b1s[:], b1_in[:, :])
            b2s = cst.tile([1, D], f32)
            nc.sync.dma_start(b2s[:], b2_in[:, :])
            b3s = cst.tile([1, D_OUT_PAD], f32)
            nc.sync.dma_start(b3s[:], b3_in[:, :])
            invd_sb = cst.tile([1, NTILES * 128], f32)
            nc.sync.dma_start(invd_sb[:], invd_in[:, :])
            dinv_sb = cst.tile([128, NTILES], f32)
            nc.sync.dma_start(dinv_sb[:], dinv_in[:, :])
            dinv2_sb = cst.tile([128, NTILES], f32)
            nc.sync.dma_start(dinv2_sb[:], dinv2_in[:, :])
            zrow = cst.tile([128, D], bf16)
            nc.vector.memset(zrow[:], 0.0)

            sh1 = dram.tile([SR, D], bf16, tag="sh1")
            sh2 = dram.tile([SR, D], bf16, tag="sh2")
            # chunked tables: contiguous Shared tensors, one AG writer each
            tchunks = []
            for ln in (1, 2):
                cs = [nc.dram_tensor(f"tb{ln}c{k}",
                                     [NCORES * (p[k + 1] - p[k]), D], bf16,
                                     addr_space="Shared")
                      for k in range(NCHUNK)]
                a0 = nc.lookup_mls(cs[0]).memorylocations[0].addr
                for k in range(1, NCHUNK):
                    ak = nc.lookup_mls(cs[k]).memorylocations[0].addr
                    exp = a0 + tbl_base[k] * D * 2
                    assert ak == exp, (ln, k, ak, exp)
                tchunks.append(cs)

            _, chunk_sets = _tile_order()
            chunk_of_tile = {}
            for ci, cs in enumerate(chunk_sets):
                for t in cs:
                    chunk_of_tile[t] = ci
            rg = [list(range(NCORES))]

            def emit_ag(sh, chunks, ck):
                if NO_AG:
                    return
                nc.gpsimd.collective_compute(
                    "AllGather", mybir.AluOpType.bypass,
                    replica_groups=rg,
                    ins=[sh[p[ck]:p[ck + 1], :].opt()],
                    outs=[chunks[ck][:, :].opt()])

            for l in range(NLAYERS):
                fo = D if l < 2 else D_OUT_PAD
                W = (w1s, w2s, w3s)[l]
                br = (b1s, b2s, b3s)[l]
                src = (t0_in[BASE:, :] if l == 0
                       else tchunks[l - 1][BASE_CHUNK][:, :])
                sh = (sh1, sh2, None)[l]
                ch_next = (tchunks[0], tchunks[1], None)[l]
                pend_ag = []
                col = 0
                done = [0] * NCHUNK   # tiles emitted per chunk
                for k, (tlist, nidx) in enumerate(calls):
                    nid = nidx + 128
                    gout = gp.tile([128, 1, GMAX], bf16, tag="g")
                    nc.gpsimd.dma_gather(
                        out_ap=gout[:, :, :nid],
                        in_ap=src,
                        idxs_ap=idx_sb[:, col:col + nid // 16],
                        num_idxs=nid, num_idxs_reg=nid, elem_size=D,
                        transpose=True, single_packet=False,
                        queue_num=k % NQ)
                    col += nid // 16
                    while (pend_ag and pend_ag[0][0] <= k
                           and k < len(calls) - 1):
                        emit_ag(sh, ch_next, pend_ag.pop(0)[1])
                    off = 0
                    for t in tlist:
                        nbt = int(nb[t])
                        pt = min(128, SHARD - t * 128)
                        agg = wrk.tile([128, 128], f32, tag="agg")
                        v = gout[:, 0, off:off + 128 * nbt].rearrange(
                            "q (j b) -> q j b", b=nbt)
                        nc.vector.tensor_reduce(agg[:], v,
                                                axis=mybir.AxisListType.X,
                                                op=mybir.AluOpType.add)
                        off += 128 * nbt
                        pst = ps.tile([128, fo], f32, tag="ps")
                        nc.tensor.matmul(pst[:], lhsT=agg[:], rhs=W[:],
                                         start=True, stop=False)
                        nc.tensor.matmul(
                            pst[:],
                            lhsT=invd_sb[0:1, t * 128:(t + 1) * 128],
                            rhs=br[0:1, :], start=False, stop=True)
                        if l < 2:
                            hsb = wrk.tile([128, D], bf16, tag="hsb")
                            nc.scalar.activation(hsb[:], pst[:], AF.Relu,
                                                 scale=dinv2_sb[:, t:t + 1])
                            nc.sync.dma_start(sh[t * 128:t * 128 + pt, :],
                                              hsb[:pt, :])
                            if t == NTILES - 1:
                                nc.sync.dma_start(sh[SHARD:SR, :],
                                                  zrow[0:SR - SHARD, :])
                            ck = chunk_of_tile[t]
                            done[ck] += 1
                            if done[ck] == CHUNK_TILES[ck]:
                                pend_ag.append((k + 2, ck))
                        else:
                            zt = wrk.tile([128, D_OUT_PAD], f32, tag="zt")
                            nc.scalar.activation(zt[:], pst[:], AF.Copy,
                                                 scale=dinv_sb[:, t:t + 1])
                            mx = wrk.tile([128, 1], f32, tag="mx")
                            nc.vector.tensor_reduce(mx[:], zt[:, :D_OUT],
                                                    axis=mybir.AxisListType.X,
                                                    op=mybir.AluOpType.max)
                            nmx = wrk.tile([128, 1], f32, tag="nmx")
                            nc.vector.tensor_scalar_mul(nmx[:], mx[:], -1.0)
                            ex = wrk.tile([128, D_OUT], f32, tag="ex")
                            se = wrk.tile([128, 1], f32, tag="se")
                            nc.scalar.activation(ex[:], zt[:, :D_OUT],
                                                 AF.Exp, bias=nmx[:, 0:1],
                                                 accum_out=se[:, 0:1])
                            lse = wrk.tile([128, 1], f32, tag="lse")
                            nc.scalar.activation(lse[:], se[:], AF.Ln)
                            ot = wrk.tile([128, D_OUT], f32, tag="ot")
                            nc.vector.tensor_scalar(
                                ot[:], zt[:, :D_OUT],
                                scalar1=mx[:, 0:1], scalar2=lse[:, 0:1],
                                op0=mybir.AluOpType.subtract,
                                op1=mybir.AluOpType.subtract)
                            nc.sync.dma_start(y_out[t * 128:t * 128 + pt, :],
                                              ot[:pt, :])
                # layer-end flush: remaining chunk AGs in completion order
                # (BASE chunk's tiles were processed last, so it flushes last)
                for (_, ck) in pend_ag:
                    emit_ag(sh, ch_next, ck)

            if DBG:
                nc.sync.dma_start(dbg_sh[:, :], sh1[:, :])
                for ck in range(NCHUNK):
                    nc.sync.dma_start(
                        dbg_tb[tbl_base[ck]:tbl_base[ck + 1], :],
                        tchunks[0][ck][:, :])

    nc.compile()
    return nc


def prepare(x, src, dst, W1, b1, W2, b2, W3, b3,
            g1, be1, m1, v1, g2, be2, m2, v2):
    bf = np.float16
    x = np.asarray(x, np.float32)
    src = np.asarray(src, np.int64)
    dst = np.asarray(dst, np.int64)
    (dinv, tid, nb, calls, idx_wrapped, dinv_t, dinv2_t, invd_row,
     shard_nodes, p, tbl_base, zid) = _preprocess(src, dst)
    nc = _build(nb, calls, p, tbl_base)

    s1 = np.asarray(g1, np.float32) / np.sqrt(np.asarray(v1, np.float32)
                                              + BN_EPS)
    s2 = np.asarray(g2, np.float32) / np.sqrt(np.asarray(v2, np.float32)
                                              + BN_EPS)
    w1p = np.asarray(W1, np.float32) * s1[None, :]
    w2p = np.asarray(W2, np.float32) * s2[None, :]
    b1p = ((np.asarray(b1, np.float32) - np.asarray(m1, np.float32)) * s1
           + np.asarray(be1, np.float32))[None, :]
    b2p = ((np.asarray(b2, np.float32) - np.asarray(m2, np.float32)) * s2
           + np.asarray(be2, np.float32))[None, :]
    w3p = np.zeros((128, D_OUT_PAD), np.float32)
    w3p[:, :D_OUT] = np.asarray(W3, np.float32)
    b3p = np.zeros((1, D_OUT_PAD), np.float32)
    b3p[0, :D_OUT] = np.asarray(b3, np.float32)

    t0 = np.zeros((TBL, D), np.float32)
    t0[tid] = x * dinv[:, None]
    t0 = t0.astype(bf)

    in_maps = []
    for c in range(NCORES):
        in_maps.append({
            "t0": t0, "idx": idx_wrapped[c],
            "w1": w1p, "w2": w2p, "w3": w3p,
            "b1r": b1p, "b2r": b2p, "b3r": b3p,
            "invd": invd_row[c],
            "dinvt": dinv_t[c], "dinv2t": dinv2_t[c],
        })
    return nc, in_maps, shard_nodes


def kernel(**inputs):
    from concourse.bass_utils import run_bass_kernel_spmd

    nc, in_maps, shard_nodes = prepare(**inputs)
    res = run_bass_kernel_spmd(nc, in_maps, core_ids=list(range(NCORES)))
    out = np.zeros((N, D_OUT), np.float32)
    for c in range(NCORES):
        out[shard_nodes[c][:SHARD]] = res.results[c]["y"]
    return out
